# revision 23
# baseline (speedup 1.0000x reference)
"""HGTConv Trainium2 kernel (8 NeuronCores, dst-sharded edge parallel).

Math: in the reference, softmax over the H=8 head axis followed by
attn.mean(axis=-1) is identically 1/8, so the whole attention branch
(K/Q projections, Wa) drops out:

    out_dst = relu( (segsum_dst(x_src[src]) @ Wbig + cnt*bbig + 8*max(cnt,1)*bout
                     + x_dst*8*max(cnt,1)) / (8*max(cnt,1)) )
    Wbig = Wv @ Wm @ Wout,  bbig = (bv @ Wm + bm) @ Wout

Sharding: each core owns a contiguous dst-node range (1/8 of users +
1/8 of games) and receives exactly the edges pointing into it; node
features are replicated (fp16), so no collectives are needed.

The HW gather pipe (SWDGE dma_gather) moves ~one 512B row per 9.2ns
regardless of row size, so execution time ~= total gathered slots x
9.2ns and everything else (HWDGE streams, PE, DVE, ACT) hides behind
it. The design therefore minimizes slots:
  - edges are packed into chunks of 128 belonging to a WINDOW of W=4
    dst tiles (512 dsts); the one-hot routing matrix M is [128, 512]
    (fp16: local-dst values up to 511 are exact; bf16 is not).
  - the source table is split into regions of <= 32768 rows (int16
    gather indices, rebased); each (window, region) owns cap chunks.
  - caps are data-driven PER WINDOW POSITION (max over the 8 cores
    only), so padding is ~2 sigma + quantization instead of 5 sigma.
One dma_gather per (window, region) run of <= 1024 idxs (single-packet
limit: 64 descriptors x 16 engines). Dummy slots gather region row 0
and are zeroed by the M matrix (ld = -1).

Scatter-add per window: PE matmuls accumulate S^T [128 f-half, 512 d]
in PSUM (one bank). The residual is added through the PE as
identity @ (x*8max(cnt,1)), so the epilogue per tile is 4 accumulate
matmuls + one fused ACT Relu with per-partition scale 1/(8max(cnt,1)).
A one-window software-pipeline lag keeps the PE from waiting on the
PSUM->SBUF copies.
"""

import math
from contextlib import ExitStack

import numpy as np
import ml_dtypes

import concourse.bass as bass
import concourse.tile as tile
import concourse.mybir as mybir
from concourse import bacc
from concourse.bass_utils import run_bass_kernel_spmd

P = 128
D = 256
FP16 = np.float16
W = 4            # dst tiles per window (W*128 <= 512 PSUM f32 bank limit)

CFG_FULL = dict(
    n_user=100000, n_game=50000, ncores=8,
    bnd_u=(0, 32768, 50000),                    # user side gathers from x_game
    bnd_g=(0, 32768, 65536, 82768, 100000),     # game side gathers from x_user
)


def _cfg_derived(cfg):
    ncores = cfg["ncores"]
    uslice = cfg["n_user"] // ncores
    gslice = cfg["n_game"] // ncores
    ut = math.ceil(uslice / P)
    gt = math.ceil(gslice / P)
    return uslice, gslice, ut, gt


# ----------------------------------------------------------------- host prep

def _edge_wr(src, dst, lo, hi, bnd):
    """Edges into [lo,hi): (src, local dst, tile, region)."""
    sel = (dst >= lo) & (dst < hi)
    s = src[sel].astype(np.int64)
    d = (dst[sel] - lo).astype(np.int64)
    t = d >> 7
    r = np.searchsorted(np.asarray(bnd), s, side="right") - 1
    return s, d, t, r


def _window_plan(cfg, n_tiles, bnd, per_core_edges):
    """Windows of W tiles with per-(window, region) chunk caps = max over
    cores. Returns list of dicts {t0, w, caps, chunk0, rb} and total chunks."""
    R = len(bnd) - 1
    t0s = list(range(0, n_tiles, W))
    nw = len(t0s)
    counts = np.zeros((nw, R), np.int64)
    for s, d, t, r in per_core_edges:
        c = np.zeros((nw, R), np.int64)
        np.add.at(c, (t // W, r), 1)
        counts = np.maximum(counts, c)
    windows = []
    chunk0 = 0
    for wi, t0 in enumerate(t0s):
        caps = [int(math.ceil(int(counts[wi, r]) / P)) for r in range(R)]
        rb = [0] + list(np.cumsum(caps))
        windows.append(dict(
            t0=t0, w=min(W, n_tiles - t0), caps=caps, rb=rb, chunk0=chunk0,
        ))
        chunk0 += rb[-1]
    return windows, chunk0


def _pack_side(src, dst, lo, hi, n_tiles, bnd, windows, n_chunks):
    """Pack one core's edges into the shared window plan.

    Returns idx16 [P, n_chunks*8] int16 (slot s at [s%16, s//16], replicated
    x8), ld [P, n_chunks] fp16 (local dst within window, dummy -> -1),
    ch [2, n_tiles*P] fp16, r8 [P, n_tiles] f32, m8 [n_tiles*P] f32.
    """
    s, d, t, r = _edge_wr(src, dst, lo, hi, bnd)
    wi = t // W
    order = np.lexsort((d, r, wi))
    s, d, t, r, wi = s[order], d[order], t[order], r[order], wi[order]

    n_slots = n_chunks * P
    idx_flat = np.zeros(n_slots, np.int64)  # dummies gather region row 0
    ld = np.full((P, n_chunks), -1.0, np.float32)

    # rank within each (window, region) run
    R = len(bnd) - 1
    key = wi * R + r
    first = np.r_[0, np.nonzero(np.diff(key))[0] + 1]
    runlen = np.diff(np.r_[first, len(key)])
    rank = np.arange(len(key)) - np.repeat(first, runlen)

    caps_arr = np.array([[wd["caps"][rr] for rr in range(R)] for wd in windows])
    chunk0_arr = np.array([wd["chunk0"] for wd in windows])
    rb_arr = np.array([[wd["rb"][rr] for rr in range(R)] for wd in windows])
    t0_arr = np.array([wd["t0"] for wd in windows])

    assert (rank < caps_arr[wi, r] * P).all(), "window/region run overflow"
    pos = chunk0_arr[wi] + rb_arr[wi, r] + rank // P
    p = rank % P
    idx_flat[pos * P + p] = s - np.asarray(bnd)[r]
    ld[p, pos] = (d - (t0_arr[wi] << 7)).astype(np.float32)

    idx16 = np.zeros((16, n_slots // 16), np.int16)
    sa = np.arange(n_slots)
    idx16[sa % 16, sa // 16] = idx_flat
    idx16 = np.tile(idx16, (8, 1)).astype(np.int16)

    cnt = np.bincount(d, minlength=n_tiles * P).astype(np.float32)
    m8 = 8.0 * np.maximum(cnt, 1.0)
    r8 = np.ascontiguousarray((1.0 / m8).reshape(n_tiles, P).T.astype(np.float32))
    return idx16, ld.astype(FP16), cnt, r8, m8


def _fold_weights(Wv, bv, Wm, bm, Wout, bout):
    Wbig = (np.float32(Wv) @ np.float32(Wm)) @ np.float32(Wout)
    bbig = (np.float32(bv) @ np.float32(Wm) + np.float32(bm)) @ np.float32(Wout)
    w = np.concatenate([Wbig, bbig[None, :], np.float32(bout)[None, :]], axis=0)
    return np.ascontiguousarray(w).astype(FP16)  # [D+2, D]


# ------------------------------------------------------------- device build

def _build(cfg, plan_u, plan_g):
    uslice, gslice, ut, gt = _cfg_derived(cfg)
    f32 = mybir.dt.float32
    fp = mybir.dt.float16
    i16 = mybir.dt.int16

    windows_u, nchunks_u = plan_u
    windows_g, nchunks_g = plan_g

    nc = bacc.Bacc(
        "TRN2",
        target_bir_lowering=False,
        debug=False,
        num_devices=cfg["ncores"],
    )

    xu_fp = nc.dram_tensor("xu_fp", [cfg["n_user"], D], fp, kind="ExternalInput")
    xg_fp = nc.dram_tensor("xg_fp", [cfg["n_game"], D], fp, kind="ExternalInput")
    ident_in = nc.dram_tensor("ident", [P, P], fp, kind="ExternalInput")

    sides = []
    for name, tiles, xsrc, bnd, (windows, n_chunks) in (
        ("u", ut, xg_fp, cfg["bnd_u"], plan_u),
        ("g", gt, xu_fp, cfg["bnd_g"], plan_g),
    ):
        side = dict(name=name, tiles=tiles, xsrc=xsrc, bnd=bnd,
                    windows=windows, n_chunks=n_chunks)
        side["xm8"] = nc.dram_tensor(f"xm8_{name}", [P, tiles * D], fp, kind="ExternalInput")
        side["idx"] = nc.dram_tensor(f"idx_{name}", [P, n_chunks * 8], i16, kind="ExternalInput")
        side["ld"] = nc.dram_tensor(f"ld_{name}", [P, n_chunks], fp, kind="ExternalInput")
        side["r8"] = nc.dram_tensor(f"r8_{name}", [P, tiles], f32, kind="ExternalInput")
        side["w"] = nc.dram_tensor(f"w_{name}", [D + 2, D], fp, kind="ExternalInput")
        side["out"] = nc.dram_tensor(f"out_{name}", [P, tiles * D], fp, kind="ExternalOutput")
        sides.append(side)

    with tile.TileContext(nc) as tc, ExitStack() as ctx:
        const = ctx.enter_context(tc.tile_pool(name="const", bufs=1))
        gx = ctx.enter_context(tc.tile_pool(name="gx", bufs=3))
        mp = ctx.enter_context(tc.tile_pool(name="mp", bufs=6))
        stp = ctx.enter_context(tc.tile_pool(name="stp", bufs=4))
        xrp = ctx.enter_context(tc.tile_pool(name="xrp", bufs=3))
        outp = ctx.enter_context(tc.tile_pool(name="outp", bufs=3))
        st_ps = ctx.enter_context(tc.tile_pool(name="st_ps", bufs=4, space="PSUM"))
        op_ps = ctx.enter_context(tc.tile_pool(name="op_ps", bufs=3, space="PSUM"))

        iota_fp = const.tile([P, W * P], fp)
        nc.gpsimd.iota(
            iota_fp[:], pattern=[[1, W * P]], base=0, channel_multiplier=0,
            allow_small_or_imprecise_dtypes=True,
        )
        ident = const.tile([P, P], fp, tag="ident", name="ident_res")
        nc.sync.dma_start(ident[:], ident_in[:])

        # init the gather buffers once: slots skipped by trailing-negative
        # gather indices must hold finite values (M zeroes them in the matmul)
        max_c = max(wd["rb"][-1] for s in sides for wd in s["windows"])
        for _ in range(3):
            Xp = gx.tile([P, max_c * D], fp, tag="gx", name="gx_win")
            nc.vector.memset(Xp[:], 0.0)

        for side in sides:
            T, NC_ = side["tiles"], side["n_chunks"]
            n = side["name"]
            side["idx_res"] = const.tile([P, NC_ * 8], i16, tag=f"idx_{n}", name=f"idx_res_{n}")
            nc.sync.dma_start(side["idx_res"][:], side["idx"][:])
            side["ld_res"] = const.tile([P, NC_], fp, tag=f"ld_{n}", name=f"ld_res_{n}")
            nc.sync.dma_start(side["ld_res"][:], side["ld"][:])
            side["r8_res"] = const.tile([P, T], f32, tag=f"r8_{n}", name=f"r8_res_{n}")
            nc.sync.dma_start(side["r8_res"][:], side["r8"][:])
            side["w0"] = const.tile([P, D], fp, tag=f"w0_{n}", name=f"w0_{n}")
            nc.sync.dma_start(side["w0"][:], side["w"][0:P, :])
            side["w1"] = const.tile([P, D], fp, tag=f"w1_{n}", name=f"w1_{n}")
            nc.sync.dma_start(side["w1"][:], side["w"][P : 2 * P, :])

        pending = None

        def finish(pend):
            side, wd, st0_ps_t, st1_ps_t, xr_g, og_g = pend
            t0, wt = wd["t0"], wd["w"]
            WP = wt * P
            st0 = stp.tile([P, W * P], fp, tag="st")
            nc.scalar.copy(st0[:, :WP], st0_ps_t[:])
            st1 = stp.tile([P, W * P], fp, tag="st")
            nc.vector.tensor_copy(st1[:, :WP], st1_ps_t[:])

            for ti in range(wt):
                t = t0 + ti
                opre = op_ps.tile([P, D], f32, tag="opre")
                # xm8 already carries x*m8 + cnt*bbig + m8*bout (host-folded)
                nc.tensor.matmul(
                    opre[:], lhsT=ident[:], rhs=xr_g[:, ti * D : (ti + 1) * D],
                    start=True, stop=False,
                )
                nc.tensor.matmul(
                    opre[:], lhsT=st0[:, ti * P : (ti + 1) * P], rhs=side["w0"][:],
                    start=False, stop=False,
                )
                nc.tensor.matmul(
                    opre[:], lhsT=st1[:, ti * P : (ti + 1) * P], rhs=side["w1"][:],
                    start=False, stop=True,
                )
                nc.scalar.activation(
                    og_g[:, ti * D : (ti + 1) * D], opre[:],
                    mybir.ActivationFunctionType.Relu,
                    scale=side["r8_res"][:, t : t + 1],
                )
            nc.sync.dma_start(
                side["out"][:, t0 * D : (t0 + wt) * D], og_g[:, : wt * D]
            )

        for side in sides:
            bnd = side["bnd"]
            for wd in side["windows"]:
                t0, wt, caps, rb, chunk0 = (
                    wd["t0"], wd["w"], wd["caps"], wd["rb"], wd["chunk0"]
                )
                WP = wt * P
                C = rb[-1]
                # gather: one call per region (<= 1024 idxs each)
                Xg = gx.tile([P, C * D], fp, tag="gx", name="gx_win")
                for r in range(len(caps)):
                    ni_all = caps[r] * P
                    done = 0
                    while done < ni_all:
                        ni = min(1024, ni_all - done)
                        c0 = rb[r] + done // P
                        slot0 = (chunk0 + c0) * P
                        out3 = Xg[:, c0 * D : (c0 + ni // P) * D] \
                            .rearrange("p (c d) -> p c d", d=D)
                        nc.gpsimd.dma_gather(
                            out_ap=out3,
                            in_ap=side["xsrc"][bnd[r] : bnd[r + 1], :],
                            idxs_ap=side["idx_res"][:, slot0 // 16 : (slot0 + ni) // 16],
                            num_idxs=ni,
                            num_idxs_reg=ni,
                            elem_size=D,
                        )
                        done += ni
                xr_g = xrp.tile([P, W * D], fp, tag="xr", name="xr_win")
                nc.sync.dma_start(xr_g[:, : wt * D], side["xm8"][:, t0 * D : (t0 + wt) * D])
                og_g = outp.tile([P, W * D], fp, tag="og", name="og_win")

                # scatter-accumulate S^T [128 f-half, wt*128] for this window
                st0_ps_t = st_ps.tile([P, WP], f32, tag="st")
                st1_ps_t = st_ps.tile([P, WP], f32, tag="st")
                for ci in range(C):
                    pos = chunk0 + ci
                    Mt = mp.tile([P, W * P], fp, tag="m")
                    nc.vector.tensor_tensor(
                        out=Mt[:, :WP],
                        in0=side["ld_res"][:, pos : pos + 1].to_broadcast([P, WP]),
                        in1=iota_fp[:, :WP],
                        op=mybir.AluOpType.is_equal,
                    )
                    s_, e_ = (ci == 0), (ci == C - 1)
                    nc.tensor.matmul(
                        st0_ps_t[:], lhsT=Xg[:, ci * D : ci * D + P], rhs=Mt[:, :WP],
                        start=s_, stop=e_,
                    )
                    nc.tensor.matmul(
                        st1_ps_t[:], lhsT=Xg[:, ci * D + P : (ci + 1) * D], rhs=Mt[:, :WP],
                        start=s_, stop=e_,
                    )

                if pending is not None:
                    finish(pending)
                pending = (side, wd, st0_ps_t, st1_ps_t, xr_g, og_g)

        finish(pending)

    nc.compile()
    return nc


_NC_CACHE = {}


def _plan_key(plan):
    windows, n_chunks = plan
    return (n_chunks, tuple((wd["t0"], wd["w"], tuple(wd["caps"])) for wd in windows))


def _get_nc(cfg, plan_u, plan_g):
    key = (tuple(sorted((k, tuple(v) if isinstance(v, (tuple, list)) else v)
                        for k, v in cfg.items())), _plan_key(plan_u), _plan_key(plan_g))
    if key not in _NC_CACHE:
        _NC_CACHE[key] = _build(cfg, plan_u, plan_g)
    return _NC_CACHE[key]


# ------------------------------------------------------------------- driver

def _run(inputs, cfg=None, trace=False, **run_kwargs):
    cfg = cfg or CFG_FULL
    uslice, gslice, ut, gt = _cfg_derived(cfg)
    ncores = cfg["ncores"]

    w_user = _fold_weights(
        inputs["Wv_game"], inputs["bv_game"], inputs["Wm_rev"], inputs["bm_rev"],
        inputs["Wout_user"], inputs["bout_user"],
    )
    w_game = _fold_weights(
        inputs["Wv_user"], inputs["bv_user"], inputs["Wm_played"], inputs["bm_played"],
        inputs["Wout_game"], inputs["bout_game"],
    )

    ei_ps = np.asarray(inputs["ei_played_src"])
    ei_pd = np.asarray(inputs["ei_played_dst"])
    ei_rs = np.asarray(inputs["ei_rev_src"])
    ei_rd = np.asarray(inputs["ei_rev_dst"])

    plan_u = _window_plan(
        cfg, ut, cfg["bnd_u"],
        [_edge_wr(ei_rs, ei_rd, k * uslice, (k + 1) * uslice, cfg["bnd_u"])
         for k in range(ncores)],
    )
    plan_g = _window_plan(
        cfg, gt, cfg["bnd_g"],
        [_edge_wr(ei_ps, ei_pd, k * gslice, (k + 1) * gslice, cfg["bnd_g"])
         for k in range(ncores)],
    )

    x_user = np.ascontiguousarray(np.float32(inputs["x_user"]))
    x_game = np.ascontiguousarray(np.float32(inputs["x_game"]))
    xu_fp = x_user.astype(FP16)
    xg_fp = x_game.astype(FP16)
    ident = np.eye(P, dtype=FP16)

    def pm_layout_m8(x_slice, cnt, m8, w, n_tiles):
        # x*m8 + cnt*bbig + m8*bout  (folds the per-tile bias matmul away)
        wf = np.float32(w)
        out = np.zeros((n_tiles * P, D), np.float32)
        n = x_slice.shape[0]
        out[:n] = x_slice * m8[:n, None] + cnt[:n, None] * wf[D] \
            + m8[:n, None] * wf[D + 1]
        return np.ascontiguousarray(
            out.reshape(n_tiles, P, D).transpose(1, 0, 2).reshape(P, n_tiles * D)
        ).astype(FP16)

    in_maps = []
    for k in range(ncores):
        idx_u, ld_u, cnt_u, r8_u, m8_u = _pack_side(
            ei_rs, ei_rd, k * uslice, (k + 1) * uslice, ut, cfg["bnd_u"],
            plan_u[0], plan_u[1],
        )
        idx_g, ld_g, cnt_g, r8_g, m8_g = _pack_side(
            ei_ps, ei_pd, k * gslice, (k + 1) * gslice, gt, cfg["bnd_g"],
            plan_g[0], plan_g[1],
        )
        in_maps.append(
            dict(
                xu_fp=xu_fp,
                xg_fp=xg_fp,
                ident=ident,
                xm8_u=pm_layout_m8(x_user[k * uslice : (k + 1) * uslice],
                                   cnt_u, m8_u, w_user, ut),
                xm8_g=pm_layout_m8(x_game[k * gslice : (k + 1) * gslice],
                                   cnt_g, m8_g, w_game, gt),
                idx_u=idx_u, ld_u=ld_u, r8_u=r8_u,
                idx_g=idx_g, ld_g=ld_g, r8_g=r8_g,
                w_u=w_user,
                w_g=w_game,
            )
        )

    nc = _get_nc(cfg, plan_u, plan_g)
    res = run_bass_kernel_spmd(nc, in_maps, list(range(ncores)), trace=trace, **run_kwargs)

    def unpm(a, n_tiles, nrows):
        a = np.float32(a)
        return a.reshape(P, n_tiles, D).transpose(1, 0, 2).reshape(n_tiles * P, D)[:nrows]

    out_user = np.concatenate(
        [unpm(res.results[k]["out_u"], ut, uslice) for k in range(ncores)], axis=0
    )
    out_game = np.concatenate(
        [unpm(res.results[k]["out_g"], gt, gslice) for k in range(ncores)], axis=0
    )
    full = np.concatenate([out_user, out_game], axis=0).astype(np.float32)
    return full, res


def kernel(**inputs) -> np.ndarray:
    out, _ = _run(inputs)
    return out
